# revision 1
# baseline (speedup 1.0000x reference)
"""Trainium2 Bass kernel for nn_ClusteringLayer (vq_codebook, Student-t assignments).

Math (ALPHA=1 makes the power a no-op):
    dist2[n,k] = ||x_n||^2 - 2 x_n.c_k + ||c_k||^2
    q = 1 / (1 + dist2)
    out = q / sum_k(q)

Device strategy (8 NeuronCores, data-parallel over N), v2 — fp16 everywhere
(tolerance is 2e-2 rel-to-max; fp16 lands at ~1e-3):

  - The host ships the ENTIRE lhsT operand pre-transposed and pre-permuted as
    one fp16 tensor lt[66, n_per]: rows 0-63 = x^T, row 64 = ones, row 65 =
    ||x||^2 per point. rhs caug[66, K]: rows 0-63 = -2 c^T, row 64 = 1+||c||^2,
    row 65 = ones. One 66-contraction matmul then yields the COMPLETE
    1 + dist2 in PSUM -- no ScalarE bias needed, no on-device transposes, no
    PSUM->SBUF weight evictions, no identity matrix.
  - lt columns are ordered (m, g, p) to match PSUM partitions, chosen so the
    output store is 4 KB contiguous per partition per macro-tile.
  - Per macro-tile (512 points): 4 matmuls fill 4 PSUM banks [128, 4, 512];
    TWO ScalarE ACTIVATE(Reciprocal) ops (one per bank pair) evict
    PSUM -> SBUF fp16. ScalarE is the pacing engine (~2.2 us/macro model).
    HW ablation: 'pair' beats one 4-bank activate and beats 4 per-bank ones.
  - Row-sums via per-subtile DVE tensor_scalar(mult, 1.0, accum_out) in fp16
    4x mode (tensor_reduce has NO fast DVE mode; tensor_scalar does).
    ACT accum_out with fp16 output measured catastrophically slow on HW - do
    not use. GpSimd fp16 tensor_scalar also catastrophic - keep GpSimd idle.
  - 1/rowsum + the fp16 4x scale pass on DVE (~2.1 us/macro).
  - Output store fp16 halves HBM write traffic (32 MiB/core).

The walrus build in this container accepts at most ONE embedded semaphore wait
per instruction; _legalize_waits() hoists extras onto standalone Drain
instructions post-scheduling (spliced into the serialized BIR).
"""

import json
import numpy as np

import concourse.bass as bass
import concourse.mybir as mybir
import concourse.tile as tile
from concourse.bass_utils import run_bass_kernel_spmd

# --------------------------------------------------------------------------- #
# Problem geometry (hardcoded per contract)
# --------------------------------------------------------------------------- #
N_CORES = 8
N_FULL, D, K = 262144, 64, 512
N_PER = N_FULL // N_CORES  # 32768 points per core
P = 128  # points per subtile (PSUM partition dim)
G = 4  # subtiles per macro-tile
KC = D + 2  # contraction rows: x(64) + ones(1) + ||x||^2(1)
F32 = mybir.dt.float32
F16 = mybir.dt.float16


def _act(nc, out, in_, func, bias=0.0, scale=1.0, accum_out=None):
    """Emit InstActivation directly (nc.scalar.activation refuses Reciprocal)."""
    eng = nc.scalar
    inputs = [eng.lower_ap(in_)]
    for arg in (bias, scale, 0.0):  # order: bias, scale, alpha
        if isinstance(arg, bass.AP):
            inputs.append(eng.lower_ap(arg))
        else:
            inputs.append(mybir.ImmediateValue(dtype=F32, value=float(arg)))
    outputs = [eng.lower_ap(out)]
    if accum_out is not None:
        outputs.append(eng.lower_ap(accum_out))
    return eng.add_instruction(
        mybir.InstActivation(
            name=nc.get_next_instruction_name(),
            func=func,
            ins=inputs,
            outs=outputs,
        )
    )


def build_nc(
    n_per=N_PER,
    repeat=1,
    act_mode="pair",
    rowsum="dve_ts",
    gps_scales=0,
    dma_split=1,
    rs_inplace=True,
    skip_store=False,
    skip_scale=False,
    skip_act=False,
    fine_psum=False,
):
    """act_mode: 'big' = one [128, G*K] activate per macro; 'pair' = two
    [128, 2K]; 'split' = per-bank. rowsum: 'dve_ts' = DVE tensor_scalar accum;
    'act_accum' = ACT accum_out (requires act_mode='split'). gps_scales: how
    many of the G scale ops run on GpSimd (rest on DVE). dma_split: output
    DMAs per macro. rs_inplace: rowsum tensor_scalar writes q in place vs to
    a scratch tile."""
    macros = n_per // (P * G)
    assert macros * P * G == n_per
    assert rowsum != "act_accum" or act_mode == "split"

    nc = bass.Bass(trn_type="TRN2")
    lt = nc.dram_tensor("lt", [KC, n_per], F16, kind="ExternalInput")
    caug = nc.dram_tensor("caug", [KC, K], F16, kind="ExternalInput")
    y = nc.dram_tensor("y", [n_per, K], F16, kind="ExternalOutput")

    # lt DRAM minor order is (m, g, p); point n = m*(P*G) + p*G + g sits at
    # column (m*G + g)*P + p, so each PSUM partition's store lands on G=4
    # consecutive DRAM rows -> 4 KB contiguous per partition per macro.
    ltv = lt[:].rearrange("kc (m g p) -> kc m g p", g=G, p=P)
    yv = y[:].rearrange("(m p g) k -> m p g k", g=G, p=P)

    RECIP = mybir.ActivationFunctionType.Reciprocal
    MULT = mybir.AluOpType.mult
    ADD = mybir.AluOpType.add

    with (
        tile.TileContext(nc) as tc,
        tc.tile_pool(name="consts", bufs=1) as consts,
        tc.tile_pool(name="outp", bufs=6) as out_pool,
        tc.tile_pool(name="small", bufs=8) as small_pool,
        tc.tile_pool(name="psS", bufs=4 if fine_psum else 2, space="PSUM") as psS_pool,
    ):
        caug_sb = consts.tile([KC, K], F16)
        nc.sync.dma_start(out=caug_sb[:], in_=caug[:])

        # Whole per-core lhsT resident in SBUF (64 KB/partition on 66
        # partitions), loaded in chunks so early macros start immediately.
        lt_sb = consts.tile([KC, macros, G, P], F16)
        n_chunks = max(1, macros // 4)
        cm = macros // n_chunks
        for c in range(n_chunks):
            nc.sync.dma_start(
                out=lt_sb[:, c * cm : (c + 1) * cm], in_=ltv[:, c * cm : (c + 1) * cm]
            )

        for _rep in range(repeat):
            for m in range(macros):
                # q = 1/(1 + dist2): PSUM -> fp16 SBUF
                out_t = out_pool.tile([P, G, K], F16)
                rs = small_pool.tile([P, G], F32)
                if fine_psum:
                    # 2-bank PSUM tiles x4 bufs: finer MM<->ACT rotation
                    assert act_mode == "pair" and not skip_act
                    for h in range(2):
                        psh = psS_pool.tile([P, 2, K], F32)
                        for j in range(2):
                            nc.tensor.matmul(
                                psh[:, j, :],
                                lt_sb[:, m, 2 * h + j, :],
                                caug_sb[:],
                                start=True,
                                stop=True,
                            )
                        _act(nc, out_t[:, 2 * h : 2 * h + 2, :], psh[:], RECIP)
                    ps = None
                else:
                    ps = psS_pool.tile([P, G, K], F32)
                    for g in range(G):
                        nc.tensor.matmul(
                            ps[:, g, :],
                            lt_sb[:, m, g, :],
                            caug_sb[:],
                            start=True,
                            stop=True,
                        )
                if fine_psum or skip_act:
                    pass
                elif act_mode == "big":
                    _act(nc, out_t[:], ps[:], RECIP)
                elif act_mode == "pair":
                    for h in range(2):
                        _act(nc, out_t[:, 2 * h : 2 * h + 2, :], ps[:, 2 * h : 2 * h + 2, :], RECIP)
                else:
                    for g in range(G):
                        _act(
                            nc,
                            out_t[:, g, :],
                            ps[:, g, :],
                            RECIP,
                            accum_out=(
                                rs[:, g : g + 1] if rowsum == "act_accum" else None
                            ),
                        )
                if skip_scale:
                    if not skip_store:
                        nc.sync.dma_start(out=yv[m], in_=out_t[:])
                    continue
                if rowsum == "dve_ts":
                    # fp16 4x tensor_scalar with accum
                    scr = None
                    if not rs_inplace:
                        scr = small_pool.tile([P, K], F16)
                    for g in range(G):
                        nc.vector.tensor_scalar(
                            out=out_t[:, g, :] if rs_inplace else scr[:],
                            in0=out_t[:, g, :],
                            scalar1=1.0,
                            scalar2=None,
                            op0=MULT,
                            op1=ADD,
                            accum_out=rs[:, g : g + 1],
                        )
                inv = small_pool.tile([P, G], F32)
                nc.vector.reciprocal(out=inv[:], in_=rs[:])
                # scale pass: split DVE (fp16 4x) / GpSimd
                gh = G // dma_split
                for h in range(dma_split):
                    for g in range(h * gh, (h + 1) * gh):
                        if g < G - gps_scales:
                            nc.vector.tensor_scalar_mul(
                                out_t[:, g, :], out_t[:, g, :], inv[:, g : g + 1]
                            )
                        else:
                            nc.gpsimd.tensor_scalar_mul(
                                out_t[:, g, :], out_t[:, g, :], inv[:, g : g + 1]
                            )
                    if not skip_store:
                        nc.sync.dma_start(
                            out=yv[m, :, h * gh : (h + 1) * gh],
                            in_=out_t[:, h * gh : (h + 1) * gh],
                        )

    _install_legalizer(nc)
    return nc


# --------------------------------------------------------------------------- #
# Wait legalizer: walrus here allows 1 embedded sync-wait per instruction.
# Hoist the rest onto preceding Drain instructions on the same engine queue.
# --------------------------------------------------------------------------- #
def _legalize_waits(bir_bytes, max_waits=1):
    bir = json.loads(bir_bytes)
    n = 0
    for fn in bir["functions"]:
        for blk in fn["blocks"]:
            out = []
            for inst in blk["instructions"]:
                si = inst.get("sync_info")
                waits = (si or {}).get("on_wait") or []
                if len(waits) > max_waits:
                    for w in waits[:-max_waits]:
                        n += 1
                        out.append(
                            {
                                "name": f"WH-{n}",
                                "opcode": "Drain",
                                "engine": inst["engine"],
                                "ins": [],
                                "outs": [],
                                "bass_is_fusable": False,
                                "sync_info": {"on_wait": [w], "on_update": []},
                            }
                        )
                    si["on_wait"] = waits[-max_waits:]
                out.append(inst)
            blk["instructions"] = out
    return json.dumps(bir).encode(), n


def _install_legalizer(nc):
    orig = nc.to_json_bytes

    def patched():
        data, n = _legalize_waits(orig())
        return data

    nc.to_json_bytes = patched


# --------------------------------------------------------------------------- #
# Host entry points
# --------------------------------------------------------------------------- #
_NC_CACHE = {}


def _get_nc(n_per=N_PER):
    if n_per not in _NC_CACHE:
        _NC_CACHE[n_per] = build_nc(n_per)
    return _NC_CACHE[n_per]


def _host_inputs(inputs, centroids):
    x = np.asarray(inputs, dtype=np.float32)
    c = np.asarray(centroids, dtype=np.float32)
    assert x.shape == (N_FULL, D) and c.shape == (K, D)
    macros = N_PER // (P * G)

    caug = np.empty((KC, K), np.float16)
    caug[0:D] = (-2.0 * c.T).astype(np.float16)
    caug[D] = (1.0 + (c * c).sum(axis=1)).astype(np.float16)
    caug[D + 1] = 1.0

    x16 = x.astype(np.float16)
    xsq16 = (x * x).sum(axis=1).astype(np.float16)

    in_maps = []
    for i in range(N_CORES):
        xs = x16[i * N_PER : (i + 1) * N_PER].reshape(macros, P, G, D)
        ltx = np.ascontiguousarray(xs.transpose(3, 0, 2, 1)).reshape(D, N_PER)
        sq = np.ascontiguousarray(
            xsq16[i * N_PER : (i + 1) * N_PER].reshape(macros, P, G).transpose(0, 2, 1)
        ).reshape(N_PER)
        lt = np.empty((KC, N_PER), np.float16)
        lt[0:D] = ltx
        lt[D] = 1.0
        lt[D + 1] = sq
        in_maps.append({"lt": lt, "caug": caug})
    return in_maps


def run(inputs, centroids, trace=False, **kwargs):
    """Run on 8 NeuronCores; returns (full_output, BassKernelResults)."""
    in_maps = _host_inputs(inputs, centroids)
    res = run_bass_kernel_spmd(
        _get_nc(), in_maps, core_ids=list(range(N_CORES)), trace=trace, **kwargs
    )
    out = np.concatenate([r["y"] for r in res.results], axis=0).astype(np.float32)
    return out, res


def kernel(inputs, centroids):
    out, _ = run(inputs, centroids, trace=False)
    return out

